# revision 46
# baseline (speedup 1.0000x reference)
"""DenseCapsule dynamic-routing kernel for 8 Trainium2 NeuronCores.

Strategy (contraction/n sharding, full batch per core):
  - x_hat is never materialized. All routing contractions are expressed
    through the shared weight W so the PE does the heavy lifting:
      s[b,(o,i)]   = sum_f W2[f,(o,i)] * (c  (*) x)[f,b]     (f = (n,j))
      t~[o][f,b]   = sum_i W2[f,(o,i)] * (g*s)[(o,i),b]
      b_inc[o][n,b]= sum_j x[f,b] * t~[o][f,b]               (block-diag PE reduce)
  - Each core owns n in [144k, 144k+144) -> f-rows 1152 = 9 chunks of 128.
    Full batch B=512 rides in the matmul free dim (N=512).
  - s partials are AllReduced across the 8 cores (iters 0,1); the final
    iteration's partial sums + squash happen on the host.
  - squash(s) = g(|s|^2) * s is folded into the t~ matmul moving operand,
    with g computed via Ln/Exp (one ACT table set, no Sqrt set switch).
"""

import sys

sys.path.insert(0, "/opt/trn_rl_repo")

import numpy as np
import ml_dtypes

import concourse.bass as bass  # noqa: F401
import concourse.tile as tile
from concourse import bacc, mybir
from concourse.bass_utils import run_bass_kernel_spmd

B, N_IN, D_IN, N_OUT, D_OUT = 512, 1152, 8, 10, 16
NCORES = 8
NLOC = N_IN // NCORES  # 144
F = NLOC * D_IN        # 1152 f-rows per core, f = 8*n_within + j
NCH = F // 128         # 9 chunks
OI = N_OUT * D_OUT     # 160
BF16 = mybir.dt.bfloat16
F32 = mybir.dt.float32
AF = mybir.ActivationFunctionType
ALU = mybir.AluOpType
bfnp = ml_dtypes.bfloat16

_built = None


def _build():
    nc = bacc.Bacc("TRN2", target_bir_lowering=False, debug=False, num_devices=NCORES)

    xT_d = nc.dram_tensor("xT", [F, B], BF16, kind="ExternalInput")
    w2_d = nc.dram_tensor("w2", [F, OI], BF16, kind="ExternalInput")
    w2t_d = nc.dram_tensor("w2t", [384, F], BF16, kind="ExternalInput")
    w2p_d = nc.dram_tensor("w2p", [F, 320], BF16, kind="ExternalInput")
    bd_d = nc.dram_tensor("bd", [128, 8 * 128], BF16, kind="ExternalInput")
    osel_d = nc.dram_tensor("osel", [384, 16], BF16, kind="ExternalInput")
    out_d = nc.dram_tensor("out", [OI, B], F32, kind="ExternalOutput")

    with tile.TileContext(nc) as tc, nc.allow_low_precision(
            reason="bf16 softmax/routing logits are within tolerance"):
        _emit(tc, nc, xT_d, w2_d, w2t_d, w2p_d, bd_d, osel_d, out_d)
    nc.compile()
    return nc


def _emit(tc, nc, xT_d, w2_d, w2t_d, w2p_d, bd_d, osel_d, out_d):
    from contextlib import ExitStack

    ctx = ExitStack()
    const = ctx.enter_context(tc.tile_pool(name="const", bufs=1))
    small = ctx.enter_context(tc.tile_pool(name="small", bufs=1))
    cxp = ctx.enter_context(tc.tile_pool(name="cx", bufs=4))
    yp = ctx.enter_context(tc.tile_pool(name="y", bufs=4))
    pp = ctx.enter_context(tc.tile_pool(name="p", bufs=4))
    tsbp = ctx.enter_context(tc.tile_pool(name="tsb", bufs=4))
    psp = ctx.enter_context(tc.tile_pool(name="psp", bufs=8, space="PSUM"))
    dram = ctx.enter_context(tc.tile_pool(name="dram", bufs=1, space="DRAM"))


    # ---- collective warmup (no deps; overlaps the prologue) ----
    wu_in = dram.tile([16, 16], F32, tag="wu_in", name="wu_in")
    wu_out = dram.tile([16, 16], F32, tag="wu_out", name="wu_out")
    nc.gpsimd.collective_compute(
        "AllReduce", ALU.add, replica_groups=[list(range(NCORES))],
        ins=[wu_in.opt()], outs=[wu_out.opt()],
    )

    # ---- load constants ----
    xT = []
    for c in range(NCH):
        t = const.tile([128, B], BF16, tag=f"xT{c}", name=f"xT{c}")
        (nc.sync if c % 2 else nc.scalar).dma_start(t[:], xT_d[128 * c:128 * (c + 1), :])
        xT.append(t)
    w2tp = []
    w2p = []
    oselg = []
    for g in range(3):
        t = const.tile([128, F], BF16, tag=f"w2tp{g}", name=f"w2tp{g}")
        (nc.sync if g % 2 else nc.scalar).dma_start(t[:], w2t_d[128 * g:128 * (g + 1), :])
        w2tp.append(t)
        t2 = const.tile([128, 16], BF16, tag=f"oselg{g}", name=f"oselg{g}")
        nc.sync.dma_start(t2[:], osel_d[128 * g:128 * (g + 1), :])
        oselg.append(t2)
    for c in range(NCH):
        t = const.tile([128, 320], BF16, tag=f"w2p{c}", name=f"w2p{c}")
        (nc.sync if c % 2 else nc.scalar).dma_start(t[:], w2p_d[128 * c:128 * (c + 1), :])
        w2p.append(t)
    bd = const.tile([128, 8 * 128], BF16, tag="bd", name="bd")
    nc.sync.dma_start(bd[:], bd_d[:])

    # ---- persistent per-routing tiles ----
    OB = N_OUT * B  # 5120
    s_red3 = []
    sTg3 = []
    grep3 = []
    sq3 = []
    s_part3 = []
    for g in range(3):
        r = small.tile([128, B], F32, tag=f"sred3{g}", name=f"sred3{g}")
        nc.gpsimd.memset(r[:], 0.0)
        s_red3.append(r)
        r = small.tile([128, B], BF16, tag=f"sTg3{g}", name=f"sTg3{g}")
        nc.gpsimd.memset(r[:], 0.0)
        sTg3.append(r)
        r = small.tile([128, B], BF16, tag=f"grep3{g}", name=f"grep3{g}")
        nc.gpsimd.memset(r[:], 0.0)
        grep3.append(r)
        r = small.tile([128, B], BF16, tag=f"sq3{g}", name=f"sq3{g}")
        nc.gpsimd.memset(r[:], 0.0)
        sq3.append(r)
        r = small.tile([128, B], F32, tag=f"spart3{g}", name=f"spart3{g}")
        s_part3.append(r)
    state_a = [small.tile([128, OB], BF16, tag=f"sta{t}", name=f"sta{t}") for t in range(2)]
    state_b = [small.tile([16, OB], BF16, tag=f"stb{t}", name=f"stb{t}") for t in range(2)]
    e_a = small.tile([128, OB], BF16, tag="e_a", name="e_a")
    e_b = small.tile([16, OB], BF16, tag="e_b", name="e_b")

    ar_in = {t: dram.tile([OI, B], F32, tag=f"arin{t}", name=f"arin{t}") for t in (0, 1)}
    ar_out = {t: dram.tile([OI, B], F32, tag=f"arout{t}", name=f"arout{t}") for t in (0, 1)}
    c_dram = dram.tile([NLOC, OB], BF16, tag="cdram", name="cdram")
    g_dram = [dram.tile([16, B], BF16, tag=f"gdram{t}", name=f"gdram{t}") for t in range(2)]

    def sl(o):
        return slice(B * o, B * (o + 1))

    # ====== iteration 0: s0 partial = sum_{f local} W2 * x, then AllReduce ==
    w2l = []
    for c in range(NCH):
        t = const.tile([128, OI], BF16, tag=f"w2l{c}", name=f"w2l{c}")
        (nc.scalar if c % 2 else nc.sync).dma_start(
            t[:], w2_d[128 * c:128 * (c + 1), :])
        w2l.append(t)
    p0a = psp.tile([128, B], F32, tag="ps", name="s0a")
    p0b = psp.tile([32, B], F32, tag="ps", name="s0b")
    for c in range(NCH):
        nc.tensor.matmul(p0a[:], w2l[c][:, 0:128], xT[c][:],
                         start=(c == 0), stop=(c == NCH - 1))
    for c in range(NCH):
        nc.tensor.matmul(p0b[:], w2l[c][:, 128:160], xT[c][:],
                         start=(c == 0), stop=(c == NCH - 1))
    s0sb_a = small.tile([128, B], F32, tag="s0sba", name="s0sba")
    s0sb_b = small.tile([32, B], F32, tag="s0sbb", name="s0sbb")
    nc.scalar.copy(s0sb_a[:], p0a[:])
    nc.scalar.copy(s0sb_b[:], p0b[:])
    nc.sync.dma_start(ar_in[0][0:128, :], s0sb_a[:])
    nc.sync.dma_start(ar_in[0][128:160, :], s0sb_b[:])
    nc.gpsimd.collective_compute(
        "AllReduce", ALU.add, replica_groups=[list(range(NCORES))],
        ins=[ar_in[0].opt()], outs=[ar_out[0].opt()],
    )
    for o in range(N_OUT):
        g, u = o // 4, o % 4
        nc.sync.dma_start(s_red3[g][32 * u:32 * u + 16, :],
                          ar_out[0][16 * o:16 * (o + 1), :])

    def g_chain(t, alpha):
        """ps_n2 <- |s|^2 per o; grep3 <- repeated ghat rows; sTg3 <- ghat*s."""
        pn2 = psp.tile([16, B], F32, tag="ps", name="n2")
        for g in range(3):
            nc.vector.tensor_mul(sq3[g][:], s_red3[g][:], s_red3[g][:])
            nc.tensor.matmul(pn2[:], oselg[g][:], sq3[g][:],
                             start=(g == 0), stop=(g == 2))
        a2 = float(alpha * alpha)
        g_ln = small.tile([16, B], F32, tag=f"gln{t}", name=f"gln{t}")
        nc.scalar.activation(g_ln[:], pn2[:], AF.Ln, scale=a2)
        g_rt = small.tile([16, B], F32, tag=f"grt{t}", name=f"grt{t}")
        nc.scalar.activation(g_rt[:], g_ln[:], AF.Exp, scale=0.5)
        # ghat = alpha * sqrt(n2) / (1 + n2); with rt = sqrt(n2)/alpha... fold
        # alpha into the denominator: (1 + a2*n2raw)/alpha = alpha*n2raw + 1/alpha
        g_d = small.tile([16, B], F32, tag=f"gd{t}", name=f"gd{t}")
        nc.vector.tensor_scalar(g_d[:], pn2[:], float(alpha), 1.0 / float(alpha),
                                ALU.mult, ALU.add)
        g_r = small.tile([16, B], F32, tag=f"gr{t}", name=f"gr{t}")
        nc.vector.reciprocal(g_r[:], g_d[:])
        g_hat = small.tile([16, B], BF16, tag=f"ghat{t}", name=f"ghat{t}")
        nc.vector.tensor_mul(g_hat[:], g_rt[:], g_r[:])
        # replicate ghat rows (o on rows) to 16-row blocks via DRAM bounce
        nc.sync.dma_start(g_dram[t][:], g_hat[:])
        for o in range(N_OUT):
            g, u = o // 4, o % 4
            nc.sync.dma_start(
                grep3[g][32 * u:32 * u + 16, :],
                g_dram[t][o:o + 1, :].broadcast_to((16, B)),
            )
        for g in range(3):
            nc.vector.tensor_mul(sTg3[g][:], grep3[g][:], s_red3[g][:])

    def agreement(t):
        """state[t] <- (t? state[t-1] : 0) + ghat (.) sum_j x*t~  (all o)."""
        for g in range(3):
            nu = 4 if g < 2 else 2
            pba = {}
            for u in range(nu):
                pba[u] = psp.tile([128, B], F32, tag="ps", name="ba")
            pbb = {}
            for u in range(nu):
                pbb[u] = psp.tile([16, B], F32, tag="ps", name="bb")
            for c in range(NCH):
                for u0 in range(0, nu, 2):
                    pts = {}
                    for u in (u0, u0 + 1):
                        if u >= nu:
                            continue
                        pt = psp.tile([128, B], F32, tag="ps", name="t")
                        nc.tensor.matmul(
                            pt[:], w2tp[g][32 * u:32 * (u + 1), 128 * c:128 * (c + 1)],
                            sTg3[g][32 * u:32 * (u + 1), :],
                            start=True, stop=True, tile_position=(32 * u, 0))
                        pts[u] = pt
                    for u in pts:
                        p = pp.tile([128, B], BF16, tag="p", name="p")
                        if u % 2 == 0:
                            tsb = tsbp.tile([128, B], BF16, tag="tsb", name="tsb")
                            nc.scalar.copy(tsb[:], pts[u][:])
                            nc.vector.tensor_mul(p[:], tsb[:], xT[c][:])
                        else:
                            nc.vector.scalar_tensor_tensor(
                                p[:], pts[u][:], 1.0, xT[c][:],
                                op0=ALU.mult, op1=ALU.mult)
                        if c < 8:
                            nc.tensor.matmul(pba[u][:], bd[:, 128 * c:128 * (c + 1)],
                                             p[:], start=(c == 0), stop=(c == 7))
                        else:
                            nc.tensor.matmul(pbb[u][:], bd[:, 0:16], p[:],
                                             start=True, stop=True)
            for u in range(nu):
                o = 4 * g + u
                if t == 0:
                    nc.scalar.copy(state_a[0][:, sl(o)], pba[u][:])
                    nc.scalar.copy(state_b[0][:, sl(o)], pbb[u][:])
                else:
                    nc.vector.scalar_tensor_tensor(
                        state_a[1][:, sl(o)], pba[u][:], 1.0,
                        state_a[0][:, sl(o)], op0=ALU.mult, op1=ALU.add)
                    nc.vector.scalar_tensor_tensor(
                        state_b[1][:, sl(o)], pbb[u][:], 1.0,
                        state_b[0][:, sl(o)], op0=ALU.mult, op1=ALU.add)

    def softmax(t):
        """e_a/e_b <- softmax over o of state[t] (written in place as c)."""
        for o in range(N_OUT):
            nc.scalar.activation(e_a[:, sl(o)], state_a[t][:, sl(o)], AF.Exp)
        nc.scalar.activation(e_b[:], state_b[t][:], AF.Exp)
        z_a = small.tile([128, B], BF16, tag=f"za{t}", name=f"za{t}")
        z_b = small.tile([16, B], BF16, tag=f"zb{t}", name=f"zb{t}")
        nc.vector.tensor_copy(z_a[:], e_a[:, sl(0)])
        nc.vector.tensor_copy(z_b[:], e_b[:, sl(0)])
        for o in range(1, N_OUT):
            nc.vector.tensor_add(z_a[:], z_a[:], e_a[:, sl(o)])
            nc.vector.tensor_add(z_b[:], z_b[:], e_b[:, sl(o)])
        zi_a = small.tile([128, B], BF16, tag=f"zia{t}", name=f"zia{t}")
        zi_b = small.tile([16, B], BF16, tag=f"zib{t}", name=f"zib{t}")
        nc.vector.reciprocal(zi_a[:], z_a[:])
        nc.vector.reciprocal(zi_b[:], z_b[:])
        ea3 = e_a[:].rearrange("p (o b) -> p o b", o=N_OUT)
        eb3 = e_b[:].rearrange("p (o b) -> p o b", o=N_OUT)
        nc.vector.tensor_mul(
            ea3, ea3, zi_a[:].unsqueeze(1).broadcast_to((128, N_OUT, B)))
        nc.vector.tensor_mul(
            eb3, eb3, zi_b[:].unsqueeze(1).broadcast_to((16, N_OUT, B)))
        nc.sync.dma_start(c_dram[0:128, :], e_a[:])
        nc.scalar.dma_start(c_dram[128:NLOC, :], e_b[:])

    def y_s_phase(itn):
        """s_part3[g] rows 32u:+16 <- sum_f W2[f,(o,:)] * (c (.) x)[f,:], o=4g+u."""
        for (w0, nw) in ((0, 8), (8, 2)):
            ngrp = nw // 4 if nw >= 4 else 1
            psos = [psp.tile([128, B], F32, tag="ps", name="so") for _ in range(max(ngrp, 1))]
            for c in range(NCH):
                cx = cxp.tile([128, nw * B], BF16, tag="cx", name="cx")
                dma_eng = nc.sync if c % 2 == 0 else nc.scalar
                dma_eng.dma_start(
                    cx[:],
                    c_dram[16 * c:16 * (c + 1),
                           B * w0:B * (w0 + nw)].unsqueeze(1).broadcast_to(
                        (16, 8, nw * B)),
                )
                for uu in range(nw):
                    o = w0 + uu
                    u = uu % 4
                    y = yp.tile([128, B], BF16, tag="y", name="y")
                    nc.vector.tensor_mul(y[:], xT[c][:], cx[:, B * uu:B * (uu + 1)])
                    nc.tensor.matmul(psos[uu // 4][32 * u:32 * (u + 1), :],
                                     w2p[c][:, 32 * o:32 * (o + 1)], y[:],
                                     start=(c == 0), stop=(c == NCH - 1),
                                     tile_position=(0, 32 * u))
            for gg in range(max(ngrp, 1)):
                nc.scalar.copy(s_part3[w0 // 4 + gg][:], psos[gg][:])

    # =====================  routing  =====================================
    g_chain(0, 0.1)
    agreement(0)
    softmax(0)
    y_s_phase(1)

    # ---- AllReduce s1 ----
    for o in range(N_OUT):
        g, u = o // 4, o % 4
        nc.sync.dma_start(ar_in[1][16 * o:16 * (o + 1), :],
                          s_part3[g][32 * u:32 * u + 16, :])
    nc.gpsimd.collective_compute(
        "AllReduce", ALU.add, replica_groups=[list(range(NCORES))],
        ins=[ar_in[1].opt()], outs=[ar_out[1].opt()],
    )
    for o in range(N_OUT):
        g, u = o // 4, o % 4
        nc.sync.dma_start(s_red3[g][32 * u:32 * u + 16, :],
                          ar_out[1][16 * o:16 * (o + 1), :])

    g_chain(1, 1.0)
    agreement(1)
    softmax(1)
    y_s_phase(2)

    # ---- write s2 partials ----
    for o in range(N_OUT):
        g, u = o // 4, o % 4
        nc.sync.dma_start(out_d[16 * o:16 * (o + 1), :],
                          s_part3[g][32 * u:32 * u + 16, :])

    ctx.close()


def _prep_inputs(x, weight):
    """Host-side layout prep. Returns per-core input maps."""
    x = np.asarray(x, dtype=np.float32)
    weight = np.asarray(weight, dtype=np.float32)
    bd_all = np.zeros((128, 8 * 128), dtype=bfnp)
    for cp in range(8):
        for p in range(128):
            bd_all[p, 128 * cp + 16 * cp + p // 8] = 1.0
    # oselg: [3][128, 16]; row p = 32u + i (i<16 live), col m = o = 4g+u
    oselg = np.zeros((3, 128, 16), dtype=bfnp)
    for g in range(3):
        for u in range(4 if g < 2 else 2):
            oselg[g, 32 * u:32 * u + 16, 4 * g + u] = 1.0
    oselg = oselg.reshape(384, 16)
    in_maps = []
    for k in range(NCORES):
        n0, n1 = NLOC * k, NLOC * (k + 1)
        xs = x[:, n0:n1, :]                      # [B, 144, 8]
        xT = np.ascontiguousarray(
            xs.transpose(1, 2, 0).reshape(F, B)).astype(bfnp)
        Wk = weight[:, n0:n1, :, :]              # [10, 144, 16, 8]
        w2 = np.ascontiguousarray(
            Wk.transpose(1, 3, 0, 2).reshape(F, OI)).astype(bfnp)
        w2t = np.ascontiguousarray(w2.T)          # [160, F]
        # w2tp: [3][128, F], rows 32u+0:16 = w2t rows of o=4g+u, rest zero
        w2tp = np.zeros((3, 128, F), dtype=bfnp)
        for g in range(3):
            for u in range(4 if g < 2 else 2):
                o = 4 * g + u
                w2tp[g, 32 * u:32 * u + 16, :] = w2t[16 * o:16 * (o + 1), :]
        w2tp = w2tp.reshape(384, F)
        # w2p: [F, 320], cols 32o+i (i<16) = w2 col 16o+i, rest zero
        w2p = np.zeros((F, 320), dtype=bfnp)
        for o in range(N_OUT):
            w2p[:, 32 * o:32 * o + 16] = w2[:, 16 * o:16 * (o + 1)]
        in_maps.append({
            "xT": xT, "w2": w2.astype(bfnp), "w2t": w2tp,
            "w2p": w2p, "bd": bd_all, "osel": oselg,
        })
    return in_maps


def _squash_np(s):
    norm = np.linalg.norm(s, axis=-1, keepdims=True)
    return (norm ** 2 / (1.0 + norm ** 2) / (norm + 1e-8)) * s


def run_spmd(x, weight, trace=False, tmpdir=None):
    global _built
    if _built is None:
        _built = _build()
    nc = _built
    in_maps = _prep_inputs(x, weight)
    res = run_bass_kernel_spmd(
        nc, in_maps, list(range(NCORES)), trace=trace, tmpdir=tmpdir)
    s2 = np.zeros((OI, B), dtype=np.float32)
    for k in range(NCORES):
        s2 += res.results[k]["out"]
    s2 = s2.reshape(N_OUT, D_OUT, B).transpose(2, 0, 1)  # [B, 10, 16]
    out = _squash_np(s2).astype(np.float32)
    return out, res


def kernel(x, weight):
    out, _ = run_spmd(x, weight)
    return out


# revision 47
# speedup vs baseline: 1.1267x; 1.1267x over previous
"""DenseCapsule dynamic-routing kernel for 8 Trainium2 NeuronCores.

Strategy (contraction/n sharding, full batch per core):
  - x_hat is never materialized. All routing contractions are expressed
    through the shared weight W so the PE does the heavy lifting:
      s[b,(o,i)]   = sum_f W2[f,(o,i)] * (c  (*) x)[f,b]     (f = (n,j))
      t~[o][f,b]   = sum_i W2[f,(o,i)] * (g*s)[(o,i),b]
      b_inc[o][n,b]= sum_j x[f,b] * t~[o][f,b]               (block-diag PE reduce)
  - Each core owns n in [144k, 144k+144) -> f-rows 1152 = 9 chunks of 128.
    Full batch B=512 rides in the matmul free dim (N=512).
  - s partials are AllReduced across the 8 cores (iters 0,1); the final
    iteration's partial sums + squash happen on the host.
  - squash(s) = g(|s|^2) * s is folded into the t~ matmul moving operand,
    with g computed via Ln/Exp (one ACT table set, no Sqrt set switch).
"""

import sys

sys.path.insert(0, "/opt/trn_rl_repo")

import numpy as np
import ml_dtypes

import concourse.bass as bass  # noqa: F401
import concourse.tile as tile
from concourse import bacc, mybir
from concourse.bass_utils import run_bass_kernel_spmd

B, N_IN, D_IN, N_OUT, D_OUT = 512, 1152, 8, 10, 16
NCORES = 8
NLOC = N_IN // NCORES  # 144
F = NLOC * D_IN        # 1152 f-rows per core, f = 8*n_within + j
NCH = F // 128         # 9 chunks
OI = N_OUT * D_OUT     # 160
BF16 = mybir.dt.bfloat16
F32 = mybir.dt.float32
AF = mybir.ActivationFunctionType
ALU = mybir.AluOpType
bfnp = ml_dtypes.bfloat16

_built = None


def _build():
    nc = bacc.Bacc("TRN2", target_bir_lowering=False, debug=False, num_devices=NCORES)

    xT_d = nc.dram_tensor("xT", [F, B], BF16, kind="ExternalInput")
    w2_d = nc.dram_tensor("w2", [F, OI], BF16, kind="ExternalInput")
    w2t_d = nc.dram_tensor("w2t", [384, F], BF16, kind="ExternalInput")
    w2p_d = nc.dram_tensor("w2p", [F, 320], BF16, kind="ExternalInput")
    bd_d = nc.dram_tensor("bd", [128, 8 * 128], BF16, kind="ExternalInput")
    osel_d = nc.dram_tensor("osel", [384, 16], BF16, kind="ExternalInput")
    out_d = nc.dram_tensor("out", [OI, B], F32, kind="ExternalOutput")

    with tile.TileContext(nc) as tc, nc.allow_low_precision(
            reason="bf16 softmax/routing logits are within tolerance"):
        _emit(tc, nc, xT_d, w2_d, w2t_d, w2p_d, bd_d, osel_d, out_d)
    nc.compile()
    return nc


def _emit(tc, nc, xT_d, w2_d, w2t_d, w2p_d, bd_d, osel_d, out_d):
    from contextlib import ExitStack

    ctx = ExitStack()
    const = ctx.enter_context(tc.tile_pool(name="const", bufs=1))
    small = ctx.enter_context(tc.tile_pool(name="small", bufs=1))
    cxp = ctx.enter_context(tc.tile_pool(name="cx", bufs=4))
    yp = ctx.enter_context(tc.tile_pool(name="y", bufs=4))
    pp = ctx.enter_context(tc.tile_pool(name="p", bufs=4))
    tsbp = ctx.enter_context(tc.tile_pool(name="tsb", bufs=4))
    psp = ctx.enter_context(tc.tile_pool(name="psp", bufs=8, space="PSUM"))
    dram = ctx.enter_context(tc.tile_pool(name="dram", bufs=1, space="DRAM"))


    # ---- collective warmup (no deps; overlaps the prologue) ----
    wu_in = dram.tile([16, 16], F32, tag="wu_in", name="wu_in")
    wu_out = dram.tile([16, 16], F32, tag="wu_out", name="wu_out")
    nc.gpsimd.collective_compute(
        "AllReduce", ALU.add, replica_groups=[list(range(NCORES))],
        ins=[wu_in.opt()], outs=[wu_out.opt()],
    )

    # ---- load constants ----
    xT = []
    for c in range(NCH):
        t = const.tile([128, B], BF16, tag=f"xT{c}", name=f"xT{c}")
        (nc.sync if c % 2 else nc.scalar).dma_start(t[:], xT_d[128 * c:128 * (c + 1), :])
        xT.append(t)
    w2tp = []
    w2p = []
    oselg = []
    for g in range(3):
        t = const.tile([128, F], BF16, tag=f"w2tp{g}", name=f"w2tp{g}")
        (nc.sync if g % 2 else nc.scalar).dma_start(t[:], w2t_d[128 * g:128 * (g + 1), :])
        w2tp.append(t)
        t2 = const.tile([128, 16], BF16, tag=f"oselg{g}", name=f"oselg{g}")
        nc.sync.dma_start(t2[:], osel_d[128 * g:128 * (g + 1), :])
        oselg.append(t2)
    for c in range(NCH):
        t = const.tile([128, 320], BF16, tag=f"w2p{c}", name=f"w2p{c}")
        (nc.sync if c % 2 else nc.scalar).dma_start(t[:], w2p_d[128 * c:128 * (c + 1), :])
        w2p.append(t)
    bd = const.tile([128, 8 * 128], BF16, tag="bd", name="bd")
    nc.sync.dma_start(bd[:], bd_d[:])

    # ---- persistent per-routing tiles ----
    OB = N_OUT * B  # 5120
    s_red3 = []
    sTg3 = []
    grep3 = []
    sq3 = []
    s_part3 = []
    for g in range(3):
        r = small.tile([128, B], F32, tag=f"sred3{g}", name=f"sred3{g}")
        nc.gpsimd.memset(r[:], 0.0)
        s_red3.append(r)
        r = small.tile([128, B], BF16, tag=f"sTg3{g}", name=f"sTg3{g}")
        nc.gpsimd.memset(r[:], 0.0)
        sTg3.append(r)
        r = small.tile([128, B], BF16, tag=f"grep3{g}", name=f"grep3{g}")
        nc.gpsimd.memset(r[:], 0.0)
        grep3.append(r)
        r = small.tile([128, B], BF16, tag=f"sq3{g}", name=f"sq3{g}")
        nc.gpsimd.memset(r[:], 0.0)
        sq3.append(r)
        r = small.tile([128, B], F32, tag=f"spart3{g}", name=f"spart3{g}")
        s_part3.append(r)
    state_a = [small.tile([128, OB], BF16, tag=f"sta{t}", name=f"sta{t}") for t in range(2)]
    state_b = [small.tile([16, OB], BF16, tag=f"stb{t}", name=f"stb{t}") for t in range(2)]
    e_a = small.tile([128, OB], BF16, tag="e_a", name="e_a")
    e_b = small.tile([16, OB], BF16, tag="e_b", name="e_b")

    ar_in = {t: dram.tile([OI, B], F32, tag=f"arin{t}", name=f"arin{t}") for t in (0, 1)}
    ar_out = {t: dram.tile([OI, B], F32, tag=f"arout{t}", name=f"arout{t}") for t in (0, 1)}
    c_dram = dram.tile([NLOC, OB], BF16, tag="cdram", name="cdram")
    g_dram = [dram.tile([16, B], BF16, tag=f"gdram{t}", name=f"gdram{t}") for t in range(2)]

    def sl(o):
        return slice(B * o, B * (o + 1))

    # ====== iteration 0: s0 partial = sum_{f local} W2 * x, then AllReduce ==
    w2l = []
    for c in range(NCH):
        t = const.tile([128, OI], BF16, tag=f"w2l{c}", name=f"w2l{c}")
        (nc.scalar if c % 2 else nc.sync).dma_start(
            t[:], w2_d[128 * c:128 * (c + 1), :])
        w2l.append(t)
    p0a = psp.tile([128, B], F32, tag="ps", name="s0a")
    p0b = psp.tile([32, B], F32, tag="ps", name="s0b")
    for c in range(NCH):
        nc.tensor.matmul(p0a[:], w2l[c][:, 0:128], xT[c][:],
                         start=(c == 0), stop=(c == NCH - 1))
    for c in range(NCH):
        nc.tensor.matmul(p0b[:], w2l[c][:, 128:160], xT[c][:],
                         start=(c == 0), stop=(c == NCH - 1))
    s0sb_a = small.tile([128, B], F32, tag="s0sba", name="s0sba")
    s0sb_b = small.tile([32, B], F32, tag="s0sbb", name="s0sbb")
    nc.scalar.copy(s0sb_a[:], p0a[:])
    nc.scalar.copy(s0sb_b[:], p0b[:])
    nc.sync.dma_start(ar_in[0][0:128, :], s0sb_a[:])
    nc.sync.dma_start(ar_in[0][128:160, :], s0sb_b[:])
    nc.gpsimd.collective_compute(
        "AllReduce", ALU.add, replica_groups=[list(range(NCORES))],
        ins=[ar_in[0].opt()], outs=[ar_out[0].opt()],
    )
    for o in range(N_OUT):
        g, u = o // 4, o % 4
        nc.sync.dma_start(s_red3[g][32 * u:32 * u + 16, :],
                          ar_out[0][16 * o:16 * (o + 1), :])

    def g_chain(t, alpha):
        """ps_n2 <- |s|^2 per o; grep3 <- repeated ghat rows; sTg3 <- ghat*s."""
        pn2 = psp.tile([16, B], F32, tag="ps", name="n2")
        for g in range(3):
            nc.vector.tensor_mul(sq3[g][:], s_red3[g][:], s_red3[g][:])
            nc.tensor.matmul(pn2[:], oselg[g][:], sq3[g][:],
                             start=(g == 0), stop=(g == 2))
        a2 = float(alpha * alpha)
        g_ln = small.tile([16, B], F32, tag=f"gln{t}", name=f"gln{t}")
        nc.scalar.activation(g_ln[:], pn2[:], AF.Ln, scale=a2)
        g_rt = small.tile([16, B], F32, tag=f"grt{t}", name=f"grt{t}")
        nc.scalar.activation(g_rt[:], g_ln[:], AF.Exp, scale=0.5)
        # ghat = alpha * sqrt(n2) / (1 + n2); with rt = sqrt(n2)/alpha... fold
        # alpha into the denominator: (1 + a2*n2raw)/alpha = alpha*n2raw + 1/alpha
        g_d = small.tile([16, B], F32, tag=f"gd{t}", name=f"gd{t}")
        nc.vector.tensor_scalar(g_d[:], pn2[:], float(alpha), 1.0 / float(alpha),
                                ALU.mult, ALU.add)
        g_r = small.tile([16, B], F32, tag=f"gr{t}", name=f"gr{t}")
        nc.vector.reciprocal(g_r[:], g_d[:])
        g_hat = small.tile([16, B], BF16, tag=f"ghat{t}", name=f"ghat{t}")
        nc.vector.tensor_mul(g_hat[:], g_rt[:], g_r[:])
        # replicate ghat rows (o on rows) to 16-row blocks via DRAM bounce
        nc.sync.dma_start(g_dram[t][:], g_hat[:])
        for o in range(N_OUT):
            g, u = o // 4, o % 4
            nc.sync.dma_start(
                grep3[g][32 * u:32 * u + 16, :],
                g_dram[t][o:o + 1, :].broadcast_to((16, B)),
            )
        for g in range(3):
            nc.vector.tensor_mul(sTg3[g][:], grep3[g][:], s_red3[g][:])

    def agreement(t):
        """state[t] <- (t? state[t-1] : 0) + ghat (.) sum_j x*t~  (all o)."""
        for g in range(3):
            nu = 4 if g < 2 else 2
            pba = {}
            for u in range(nu):
                pba[u] = psp.tile([128, B], F32, tag="ps", name="ba")
            pbb = {}
            for u in range(nu):
                pbb[u] = psp.tile([16, B], F32, tag="ps", name="bb")
            for c in range(NCH):
                for u0 in range(0, nu, 2):
                    pts = {}
                    for u in (u0, u0 + 1):
                        if u >= nu:
                            continue
                        pt = psp.tile([128, B], F32, tag="ps", name="t")
                        nc.tensor.matmul(
                            pt[:], w2tp[g][32 * u:32 * (u + 1), 128 * c:128 * (c + 1)],
                            sTg3[g][32 * u:32 * (u + 1), :],
                            start=True, stop=True, tile_position=(32 * u, 0))
                        pts[u] = pt
                    for u in pts:
                        tsb = tsbp.tile([128, B], BF16, tag="tsb", name="tsb")
                        if u % 2 == 0:
                            nc.scalar.copy(tsb[:], pts[u][:])
                        else:
                            nc.vector.tensor_copy(tsb[:], pts[u][:])
                        p = pp.tile([128, B], BF16, tag="p", name="p")
                        nc.vector.tensor_mul(p[:], tsb[:], xT[c][:])
                        if c < 8:
                            nc.tensor.matmul(pba[u][:], bd[:, 128 * c:128 * (c + 1)],
                                             p[:], start=(c == 0), stop=(c == 7))
                        else:
                            nc.tensor.matmul(pbb[u][:], bd[:, 0:16], p[:],
                                             start=True, stop=True)
            for u in range(nu):
                o = 4 * g + u
                if t == 0:
                    nc.scalar.copy(state_a[0][:, sl(o)], pba[u][:])
                    nc.scalar.copy(state_b[0][:, sl(o)], pbb[u][:])
                else:
                    nc.vector.scalar_tensor_tensor(
                        state_a[1][:, sl(o)], pba[u][:], 1.0,
                        state_a[0][:, sl(o)], op0=ALU.mult, op1=ALU.add)
                    nc.vector.scalar_tensor_tensor(
                        state_b[1][:, sl(o)], pbb[u][:], 1.0,
                        state_b[0][:, sl(o)], op0=ALU.mult, op1=ALU.add)

    def softmax(t):
        """e_a/e_b <- softmax over o of state[t] (written in place as c)."""
        for o in range(N_OUT):
            nc.scalar.activation(e_a[:, sl(o)], state_a[t][:, sl(o)], AF.Exp)
        nc.scalar.activation(e_b[:], state_b[t][:], AF.Exp)
        z_a = small.tile([128, B], BF16, tag=f"za{t}", name=f"za{t}")
        z_b = small.tile([16, B], BF16, tag=f"zb{t}", name=f"zb{t}")
        nc.vector.tensor_copy(z_a[:], e_a[:, sl(0)])
        nc.vector.tensor_copy(z_b[:], e_b[:, sl(0)])
        for o in range(1, N_OUT):
            nc.vector.tensor_add(z_a[:], z_a[:], e_a[:, sl(o)])
            nc.vector.tensor_add(z_b[:], z_b[:], e_b[:, sl(o)])
        zi_a = small.tile([128, B], BF16, tag=f"zia{t}", name=f"zia{t}")
        zi_b = small.tile([16, B], BF16, tag=f"zib{t}", name=f"zib{t}")
        nc.vector.reciprocal(zi_a[:], z_a[:])
        nc.vector.reciprocal(zi_b[:], z_b[:])
        ea3 = e_a[:].rearrange("p (o b) -> p o b", o=N_OUT)
        eb3 = e_b[:].rearrange("p (o b) -> p o b", o=N_OUT)
        nc.vector.tensor_mul(
            ea3, ea3, zi_a[:].unsqueeze(1).broadcast_to((128, N_OUT, B)))
        nc.vector.tensor_mul(
            eb3, eb3, zi_b[:].unsqueeze(1).broadcast_to((16, N_OUT, B)))
        nc.sync.dma_start(c_dram[0:128, :], e_a[:])
        nc.scalar.dma_start(c_dram[128:NLOC, :], e_b[:])

    def y_s_phase(itn):
        """s_part3[g] rows 32u:+16 <- sum_f W2[f,(o,:)] * (c (.) x)[f,:], o=4g+u."""
        for (w0, nw) in ((0, 8), (8, 2)):
            ngrp = nw // 4 if nw >= 4 else 1
            psos = [psp.tile([128, B], F32, tag="ps", name="so") for _ in range(max(ngrp, 1))]
            for c in range(NCH):
                cx = cxp.tile([128, nw * B], BF16, tag="cx", name="cx")
                dma_eng = nc.sync if c % 2 == 0 else nc.scalar
                dma_eng.dma_start(
                    cx[:],
                    c_dram[16 * c:16 * (c + 1),
                           B * w0:B * (w0 + nw)].unsqueeze(1).broadcast_to(
                        (16, 8, nw * B)),
                )
                for uu in range(nw):
                    o = w0 + uu
                    u = uu % 4
                    y = yp.tile([128, B], BF16, tag="y", name="y")
                    nc.vector.tensor_mul(y[:], xT[c][:], cx[:, B * uu:B * (uu + 1)])
                    nc.tensor.matmul(psos[uu // 4][32 * u:32 * (u + 1), :],
                                     w2p[c][:, 32 * o:32 * (o + 1)], y[:],
                                     start=(c == 0), stop=(c == NCH - 1),
                                     tile_position=(0, 32 * u))
            for gg in range(max(ngrp, 1)):
                nc.scalar.copy(s_part3[w0 // 4 + gg][:], psos[gg][:])

    # =====================  routing  =====================================
    g_chain(0, 0.1)
    agreement(0)
    softmax(0)
    y_s_phase(1)

    # ---- AllReduce s1 ----
    for o in range(N_OUT):
        g, u = o // 4, o % 4
        nc.sync.dma_start(ar_in[1][16 * o:16 * (o + 1), :],
                          s_part3[g][32 * u:32 * u + 16, :])
    nc.gpsimd.collective_compute(
        "AllReduce", ALU.add, replica_groups=[list(range(NCORES))],
        ins=[ar_in[1].opt()], outs=[ar_out[1].opt()],
    )
    for o in range(N_OUT):
        g, u = o // 4, o % 4
        nc.sync.dma_start(s_red3[g][32 * u:32 * u + 16, :],
                          ar_out[1][16 * o:16 * (o + 1), :])

    g_chain(1, 1.0)
    agreement(1)
    softmax(1)
    y_s_phase(2)

    # ---- write s2 partials ----
    for o in range(N_OUT):
        g, u = o // 4, o % 4
        nc.sync.dma_start(out_d[16 * o:16 * (o + 1), :],
                          s_part3[g][32 * u:32 * u + 16, :])

    ctx.close()


def _prep_inputs(x, weight):
    """Host-side layout prep. Returns per-core input maps."""
    x = np.asarray(x, dtype=np.float32)
    weight = np.asarray(weight, dtype=np.float32)
    bd_all = np.zeros((128, 8 * 128), dtype=bfnp)
    for cp in range(8):
        for p in range(128):
            bd_all[p, 128 * cp + 16 * cp + p // 8] = 1.0
    # oselg: [3][128, 16]; row p = 32u + i (i<16 live), col m = o = 4g+u
    oselg = np.zeros((3, 128, 16), dtype=bfnp)
    for g in range(3):
        for u in range(4 if g < 2 else 2):
            oselg[g, 32 * u:32 * u + 16, 4 * g + u] = 1.0
    oselg = oselg.reshape(384, 16)
    in_maps = []
    for k in range(NCORES):
        n0, n1 = NLOC * k, NLOC * (k + 1)
        xs = x[:, n0:n1, :]                      # [B, 144, 8]
        xT = np.ascontiguousarray(
            xs.transpose(1, 2, 0).reshape(F, B)).astype(bfnp)
        Wk = weight[:, n0:n1, :, :]              # [10, 144, 16, 8]
        w2 = np.ascontiguousarray(
            Wk.transpose(1, 3, 0, 2).reshape(F, OI)).astype(bfnp)
        w2t = np.ascontiguousarray(w2.T)          # [160, F]
        # w2tp: [3][128, F], rows 32u+0:16 = w2t rows of o=4g+u, rest zero
        w2tp = np.zeros((3, 128, F), dtype=bfnp)
        for g in range(3):
            for u in range(4 if g < 2 else 2):
                o = 4 * g + u
                w2tp[g, 32 * u:32 * u + 16, :] = w2t[16 * o:16 * (o + 1), :]
        w2tp = w2tp.reshape(384, F)
        # w2p: [F, 320], cols 32o+i (i<16) = w2 col 16o+i, rest zero
        w2p = np.zeros((F, 320), dtype=bfnp)
        for o in range(N_OUT):
            w2p[:, 32 * o:32 * o + 16] = w2[:, 16 * o:16 * (o + 1)]
        in_maps.append({
            "xT": xT, "w2": w2.astype(bfnp), "w2t": w2tp,
            "w2p": w2p, "bd": bd_all, "osel": oselg,
        })
    return in_maps


def _squash_np(s):
    norm = np.linalg.norm(s, axis=-1, keepdims=True)
    return (norm ** 2 / (1.0 + norm ** 2) / (norm + 1e-8)) * s


def run_spmd(x, weight, trace=False, tmpdir=None):
    global _built
    if _built is None:
        _built = _build()
    nc = _built
    in_maps = _prep_inputs(x, weight)
    res = run_bass_kernel_spmd(
        nc, in_maps, list(range(NCORES)), trace=trace, tmpdir=tmpdir)
    s2 = np.zeros((OI, B), dtype=np.float32)
    for k in range(NCORES):
        s2 += res.results[k]["out"]
    s2 = s2.reshape(N_OUT, D_OUT, B).transpose(2, 0, 1)  # [B, 10, 16]
    out = _squash_np(s2).astype(np.float32)
    return out, res


def kernel(x, weight):
    out, _ = run_spmd(x, weight)
    return out
